# revision 39
# baseline (speedup 1.0000x reference)
"""DigitCaps dynamic-routing kernel for 8 TRN2 NeuronCores.

Math refactor (u_hat is NEVER materialized - it would be 189 MB):
  u_hat[b,r,c,d] = sum_i W[r,c,d,i] * u[b,r,i]
  softmax over r without max-subtraction (b_ij values are O(1)):
      c_ij[r,c,d] = exp(b[r,c,d]) / Z[c,d],  Z = sum_r exp(b)
  s[b,c,d]  = (sum_{r,i} (exp(b) * W)[r,c,d,i] u[b,r,i]) / Z[c,d]
  v = squash(s) = s*|s| / (1 + s^2)      [exact modulo the eps term]
    = w*|w| / (Z^2 + w^2) with w = s*Z   -> no division by Z needed
  b += (1/B) sum_b t[b,r,c] v[b,c,d],  t[b,r,c] = sum_i (sum_d W) u[b,r,i]
       (t is iteration-invariant -> computed once, during the first AllReduce)

Sharding: routes (R=1152) split across 8 cores (144 each). Iterations 0/1
end in an fp16 AllReduce of (partial s' || partial Z); the final iteration
uses ReduceScatter instead, so each core squashes and emits 1/8 of the
batch rows and the host reassembles.

Per-core layout: contraction dim K = (r_local, i) = 1152 = 9 chunks of 128.
All matmuls put K on partitions; host pre-permutes u and W into that layout
(pure permutation + fp16 cast - no reference compute happens on host).
"""

import os
import numpy as np

B, R, C, D, I = 256, 1152, 10, 16, 8
CD = C * D                 # 160
NCORES = 8
RL = R // NCORES           # routes per core
NCHUNK = RL * I // 128     # K-chunks of 128
NG = 3                     # chunks per group (b_ij tile partition packing)
GRPS = NCHUNK // NG        # chunk groups
NITER = 3
EPS = 1e-5
PSLC = 128 // NCORES       # partitions per core after ReduceScatter

_CACHE = {}


def _build_program():
    from contextlib import ExitStack

    import concourse.bass as bass
    import concourse.bacc as bacc
    import concourse.mybir as mybir
    import concourse.tile as tile

    f32 = mybir.dt.float32
    f16 = mybir.dt.float16
    AF = mybir.ActivationFunctionType
    ALU = mybir.AluOpType

    nc = bacc.Bacc(None, num_devices=NCORES)

    # One fused input parameter -> one DMA -> one DMA semaphore, so no PE
    # instruction ever needs two sync waits (codegen limit on S3_LW).
    # Layout: [0:2304] uT | [2304:3744] Wt | [3744:3904] mask | [3904:4288] expand
    DW = NCHUNK * B + NCHUNK * CD + CD + NG * 128
    data_d = nc.declare_dram_parameter("data", [128, DW], f16, isOutput=False)
    out_d = nc.declare_dram_parameter("out", [2 * PSLC, CD], f32, isOutput=True)

    rgroups = [list(range(NCORES))]

    with tile.TileContext(nc) as tc, ExitStack() as ctx:
        singles = ctx.enter_context(tc.tile_pool(name="singles", bufs=1))
        wcpool = ctx.enter_context(tc.tile_pool(name="wc", bufs=3))
        stpool = ctx.enter_context(tc.tile_pool(name="stage", bufs=2))
        work = ctx.enter_context(tc.tile_pool(name="work", bufs=8))
        ps_s = ctx.enter_context(tc.tile_pool(name="ps_s", bufs=1, space="PSUM"))
        ps_e = ctx.enter_context(tc.tile_pool(name="ps_e", bufs=2, space="PSUM"))
        ps_z = ctx.enter_context(tc.tile_pool(name="ps_z", bufs=1, space="PSUM"))
        ps_b = ctx.enter_context(tc.tile_pool(name="ps_b", bufs=2, space="PSUM"))
        ps_j = ctx.enter_context(tc.tile_pool(name="ps_j", bufs=1, space="PSUM"))
        dram = ctx.enter_context(tc.tile_pool(name="dram", bufs=1, space="DRAM"))

        cc = []
        for it in range(NITER):
            w = 2 * CD if it == 0 else 3 * CD
            ci = dram.tile([128, w], f16, tag=f"cc_in{it}", name=f"cc_in{it}")
            po = PSLC if it == NITER - 1 else 128   # final collective is RS
            co = dram.tile([po, w], f16, tag=f"cc_out{it}", name=f"cc_out{it}")
            cc.append((ci, co, w))


        sb_data = singles.tile([128, DW], f16, tag="data")
        o_uT, o_Wt = 0, NCHUNK * B
        o_mk, o_ex = o_Wt + NCHUNK * CD, o_Wt + NCHUNK * CD + CD
        # uT lands in its own DMA so the iteration-0 matmuls don't wait for
        # the Wt/mask/expand bytes. (Do NOT split further: each dma_start
        # costs ~0.6us of serialized issue time on the Sync queue.)
        nc.sync.dma_start(out=sb_data[:, 0:o_Wt], in_=data_d[:, 0:o_Wt])
        nc.sync.dma_start(out=sb_data[:, o_Wt:DW], in_=data_d[:, o_Wt:DW])
        sb_uT = sb_data[:, o_uT:o_uT + NCHUNK * B]
        sb_Wt = sb_data[:, o_Wt:o_Wt + NCHUNK * CD]
        sb_mask = sb_data[:, o_mk:o_mk + CD]
        sb_ex = sb_data[0:48, o_ex:o_ex + NG * 128]

        # PE warm-up while the input DMA is in flight (HAM clock gate starts
        # the PE at half clock) so the iteration-0 matmuls run at full rate.
        sb_ones = singles.tile([48, 128], f16, tag="ones")
        nc.vector.memset(sb_ones, 1.0)
        for _ in range(14):
            pj = ps_j.tile([128, 16], f32, tag="junk")
            nc.tensor.matmul(
                pj, sb_ones, sb_ones[:, 0:16], start=True, stop=True,
            )

        # ---- iteration-0 s' partials first, so AllReduce #0 launches ASAP
        st = [ps_s.tile([128, CD], f32, tag=f"s{bh}", name=f"s{bh}") for bh in range(2)]
        for k in range(NCHUNK):
            for bh in range(2):
                nc.tensor.matmul(
                    st[bh],
                    sb_uT[:, k * B + bh * 128: k * B + (bh + 1) * 128],
                    sb_Wt[:, k * CD:(k + 1) * CD],
                    start=(k == 0), stop=(k == NCHUNK - 1),
                )
        ci0, co0, w0 = cc[0]
        stage0 = stpool.tile([128, w0], f16, tag="stage")
        for bh in range(2):
            nc.vector.tensor_copy(out=stage0[:, bh * CD:(bh + 1) * CD], in_=st[bh])
        nc.sync.dma_start(out=ci0[:], in_=stage0)
        nc.gpsimd.collective_compute(
            "AllReduce", mybir.AluOpType.add,
            replica_groups=rgroups, ins=[ci0.opt()], outs=[co0.opt()],
        )

        # ---- one-time prep, overlapped with the AllReduce-0 wait window
        # Wd[(rp,i), (k,c)] = (1/B) * sum_d Wt   (t pre-scaled by 1/B here)
        sb_Wd = singles.tile([128, NCHUNK * C], f32, tag="Wd")
        for k in range(NCHUNK):
            nc.vector.reduce_sum(
                out=sb_Wd[:, k * C:(k + 1) * C],
                in_=sb_Wt[:, k * CD:(k + 1) * CD].rearrange("p (c d) -> p c d", d=D),
                axis=mybir.AxisListType.X,
            )
        nc.vector.tensor_scalar_mul(sb_Wd, sb_Wd, 1.0 / B)

        # Block-diagonal Wd for the t matmul, built in one full-partition op:
        # Wdbd[p, k*CD + rp*C + c] = Wd[p,(k,c)] * mask[p, rp*C + c]
        # where mask[p, rp*C+c] = (rp == p//8). Zero-stride APs broadcast
        # Wd over rp and the mask over k.
        sb_Wdbd = singles.tile([128, NCHUNK * CD], f16, tag="Wdbd")
        wd_b = bass.AP(
            tensor=sb_Wd.tensor, offset=sb_Wd.offset,
            ap=[sb_Wd.ap[0], [C, NCHUNK], [0, 16], [1, C]],
        )
        mk_b = bass.AP(
            tensor=sb_mask.tensor, offset=sb_mask.offset,
            ap=[sb_mask.ap[0], [0, NCHUNK], [C, 16], [1, C]],
        )
        nc.vector.tensor_mul(
            sb_Wdbd.rearrange("p (k rp c) -> p k rp c", rp=16, c=C), wd_b, mk_b
        )

        # t[b, (k, rp, c)] = sum_i Wd[(rp,i),(k,c)] u[b, r(k,rp), i]
        sb_t = [singles.tile([128, NCHUNK * CD], f16, tag=f"t{bh}", name=f"t{bh}")
                for bh in range(2)]
        for k in range(NCHUNK):
            for bh in range(2):
                pt = ps_e.tile([128, CD], f32, tag="pe", name="pt")
                nc.tensor.matmul(
                    pt,
                    sb_uT[:, k * B + bh * 128: k * B + (bh + 1) * 128],
                    sb_Wdbd[:, k * CD:(k + 1) * CD],
                    start=True, stop=True,
                )
                nc.vector.tensor_copy(out=sb_t[bh][:, k * CD:(k + 1) * CD], in_=pt)

        # b_ij tile: partitions (j, rp) with j = chunk % 3, free (g, c, d)
        sb_b = singles.tile([48, GRPS * CD], f32, tag="b")
        nc.vector.memset(sb_b, 0.0)
        sb_E = singles.tile([48, GRPS * CD], f16, tag="E")
        sb_vb = singles.tile([128, 2 * CD], f16, tag="vb")

        for it in range(NITER):
            ci, co, w = cc[it]
            final = it == NITER - 1

            # ---- collect this iteration's reduced result. Issued from
            # gpsimd: it owns the collective, so it unblocks at completion
            # with no cross-engine semaphore hop before the DMA fires.
            red = stpool.tile([PSLC if final else 128, w],
                              f16, tag="red", name=f"red{it}")
            nc.gpsimd.dma_start(out=red, in_=co[:])
            P = PSLC if final else 128

            if not final:
                # PE warm-up: the HAM clock gate halves the PE clock after
                # ~3.4us idle, and the PE always idles through a collective.
                # These dummy matmuls depend on `red`, so they run during the
                # squash window and the real b-update matmuls start warm.
                for _ in range(22):
                    pj = ps_j.tile([64, 16], f32, tag="junk")
                    nc.tensor.matmul(
                        pj, red[:, 0:64], red[:, 0:16], start=True, stop=True,
                    )

            # ---- v = squash(s) = w*|w| / (Z^2 + w^2), w = red s'-part
            if it > 0:
                z2 = work.tile([P, CD], f32, tag="z2", name=f"z2{it}")
                nc.vector.tensor_mul(z2, red[:, 2 * CD:3 * CD], red[:, 2 * CD:3 * CD])
            if final:
                # full-width (tiny [16,*] tiles are overhead-bound, so fewer
                # ops beat earlier halves) + one strided output DMA
                sb_out = work.tile([P, 2 * CD], f32, tag="vout", name="vout")
                wv = red[:, 0:2 * CD]
                aw = work.tile([P, 2 * CD], f32, tag="aw", name="awf")
                nc.vector.scalar_tensor_tensor(
                    out=aw, in0=wv, scalar=-1.0, in1=wv,
                    op0=ALU.mult, op1=ALU.max,
                )
                num = work.tile([P, 2 * CD], f32, tag="num", name="numf")
                nc.vector.tensor_mul(num, aw, wv)
                den = work.tile([P, 2 * CD], f32, tag="den", name="denf")
                nc.vector.tensor_mul(den, wv, wv)
                z2b = bass.AP(tensor=z2.tensor, offset=z2.offset,
                              ap=[z2.ap[0], [0, 2], [1, CD]])
                nc.vector.tensor_add(
                    den.rearrange("p (h f) -> p h f", f=CD),
                    den.rearrange("p (h f) -> p h f", f=CD),
                    z2b,
                )
                rcp = work.tile([P, 2 * CD], f32, tag="rcp", name="rcpf")
                nc.vector.reciprocal_approx_fast(out=rcp, in_=den)
                nc.vector.tensor_mul(sb_out, num, rcp)
                nc.sync.dma_start(
                    out=out_d[:].rearrange("(h p) f -> p h f", h=2),
                    in_=sb_out.rearrange("p (h f) -> p h f", f=CD),
                )
                break
            wv = red[:, 0:2 * CD]
            aw = work.tile([P, 2 * CD], f32, tag="aw", name=f"aw{it}")
            nc.vector.scalar_tensor_tensor(
                out=aw, in0=wv, scalar=-1.0, in1=wv, op0=ALU.mult, op1=ALU.max,
            )
            num = work.tile([P, 2 * CD], f32, tag="num", name=f"num{it}")
            nc.vector.tensor_mul(num, aw, wv)
            den = work.tile([P, 2 * CD], f32, tag="den", name=f"den{it}")
            nc.vector.tensor_mul(den, wv, wv)
            if it == 0:
                nc.vector.tensor_scalar_add(den, den, float(R) * float(R))
            else:
                z2b = bass.AP(tensor=z2.tensor, offset=z2.offset,
                              ap=[z2.ap[0], [0, 2], [1, CD]])
                nc.vector.tensor_add(
                    den.rearrange("p (h f) -> p h f", f=CD),
                    den.rearrange("p (h f) -> p h f", f=CD),
                    z2b,
                )
            rcp = work.tile([P, 2 * CD], f32, tag="rcp", name=f"rcp{it}")
            nc.vector.reciprocal_approx_fast(out=rcp, in_=den)
            nc.vector.tensor_mul(sb_vb, num, rcp)

            # ---- next-iteration state, pipelined per chunk-group g:
            # b[g] += (1/B) sum_b t*v ; E[g] = exp(b[g]) ; Z += sum E[g] ;
            # s' += uT^T (Wt * expand(E[g])) for the 3 chunks of g
            ci1, co1, w1 = cc[it + 1]
            pz = ps_z.tile([128, CD], f32, tag="pz")
            st = [ps_s.tile([128, CD], f32, tag=f"s{bh}", name=f"s{bh}")
                  for bh in range(2)]
            t_r = [sb_t[bh].rearrange("p (k rp c) -> p k rp c", rp=16, c=C)
                   for bh in range(2)]
            for g in range(GRPS):
                pb = ps_b.tile([48, CD], f32, tag="pb")
                for c in range(C):
                    for bh in range(2):
                        nc.tensor.matmul(
                            pb[:, c * D:(c + 1) * D],
                            t_r[bh][:, g * NG:(g + 1) * NG, :, c],
                            sb_vb[:, bh * CD + c * D:bh * CD + (c + 1) * D],
                            start=(bh == 0), stop=(bh == 1),
                        )
                nc.vector.tensor_add(
                    sb_b[:, g * CD:(g + 1) * CD],
                    sb_b[:, g * CD:(g + 1) * CD],
                    pb,
                )
                nc.scalar.activation(
                    out=sb_E[:, g * CD:(g + 1) * CD],
                    in_=sb_b[:, g * CD:(g + 1) * CD],
                    func=AF.Exp,
                )
                nc.tensor.matmul(
                    pz, sb_ones, sb_E[:, g * CD:(g + 1) * CD],
                    start=(g == 0), stop=(g == GRPS - 1),
                )
                pe = ps_e.tile([128, NG * CD], f32, tag="pe")
                for j in range(NG):
                    nc.tensor.matmul(
                        pe[:, j * CD:(j + 1) * CD],
                        sb_ex[:, j * 128:(j + 1) * 128],
                        sb_E[:, g * CD:(g + 1) * CD],
                        start=True, stop=True,
                    )
                rhs = wcpool.tile([128, NG * CD], f16, tag="wc")
                nc.vector.tensor_mul(
                    rhs, sb_Wt[:, g * NG * CD:(g + 1) * NG * CD], pe
                )
                for j in range(NG):
                    k = g * NG + j
                    for bh in range(2):
                        nc.tensor.matmul(
                            st[bh],
                            sb_uT[:, k * B + bh * 128: k * B + (bh + 1) * 128],
                            rhs[:, j * CD:(j + 1) * CD],
                            start=(k == 0), stop=(k == NCHUNK - 1),
                        )

            # ---- stage partials (s' || Z) and launch the next collective
            # (Z first: pz completes before the last s' matmuls)
            stage = stpool.tile([128, w1], f16, tag="stage", name=f"stage{it}")
            nc.vector.tensor_copy(out=stage[:, 2 * CD:3 * CD], in_=pz)
            for bh in range(2):
                nc.vector.tensor_copy(out=stage[:, bh * CD:(bh + 1) * CD], in_=st[bh])
            nc.sync.dma_start(out=ci1[:], in_=stage)
            kind = "ReduceScatter" if it + 1 == NITER - 1 else "AllReduce"
            nc.gpsimd.collective_compute(
                kind, mybir.AluOpType.add,
                replica_groups=rgroups, ins=[ci1.opt()], outs=[co1.opt()],
            )

    nc.compile()
    return nc


def _host_inputs(u, W):
    """Pure-permutation host prep: per-core (r,i)-major layouts, fp16."""
    u = np.ascontiguousarray(u, dtype=np.float32)
    W = np.ascontiguousarray(W, dtype=np.float32)
    expand = np.zeros((48, NG * 128), dtype=np.float16)
    for j in range(NG):
        for p in range(128):
            expand[16 * j + p // 8, j * 128 + p] = 1.0
    mask = np.zeros((128, CD), dtype=np.float16)
    for p in range(128):
        mask[p, (p // 8) * C:(p // 8) * C + C] = 1.0
    DW = NCHUNK * B + NCHUNK * CD + CD + NG * 128
    o_uT, o_Wt = 0, NCHUNK * B
    o_mk, o_ex = o_Wt + NCHUNK * CD, o_Wt + NCHUNK * CD + CD
    in_maps = []
    for ci in range(NCORES):
        rs = ci * RL
        usl = u[:, rs:rs + RL, :].reshape(B, RL * I).T          # (1152, 256)
        uTd = usl.reshape(NCHUNK, 128, B).transpose(1, 0, 2).reshape(128, NCHUNK * B)
        wsl = W[rs:rs + RL].transpose(0, 3, 1, 2).reshape(RL * I, CD)
        Wtd = wsl.reshape(NCHUNK, 128, CD).transpose(1, 0, 2).reshape(128, NCHUNK * CD)
        data = np.zeros((128, DW), dtype=np.float16)
        data[:, o_uT:o_uT + NCHUNK * B] = uTd.astype(np.float16)
        data[:, o_Wt:o_Wt + NCHUNK * CD] = Wtd.astype(np.float16)
        data[:, o_mk:o_mk + CD] = mask
        data[:48, o_ex:o_ex + NG * 128] = expand
        in_maps.append({"data": data})
    return in_maps


def _install_profile_hook():
    """Recreate the missing antenv.axon_hooks NTFF-profile hook (dev only)."""
    import contextlib
    import ctypes
    import sys
    import types

    try:
        from antenv.axon_hooks import get_axon_ntff_profile_hook  # noqa: F401
        return
    except ImportError:
        pass

    mod = types.ModuleType("antenv.axon_hooks")
    holder = {}
    mod.set_axon_ntff_profile_hook = lambda h: holder.__setitem__("h", h)
    mod.get_axon_ntff_profile_hook = lambda: holder.get("h")
    import antenv

    sys.modules["antenv.axon_hooks"] = mod
    antenv.axon_hooks = mod

    so_path = "/opt/axon/libaxon_pjrt.so"
    lib = ctypes.CDLL(so_path)
    if not hasattr(lib, "axon_start_nrt_profile"):
        return
    lib.axon_start_nrt_profile.argtypes = [
        ctypes.POINTER(ctypes.c_int64),
        ctypes.c_size_t,
    ]
    lib.axon_start_nrt_profile.restype = ctypes.c_int64
    lib.axon_stop_nrt_profile.argtypes = [ctypes.c_char_p]
    lib.axon_stop_nrt_profile.restype = ctypes.c_int64

    @contextlib.contextmanager
    def _hook(output_dir, device_ids):
        import jax

        jax.devices()
        if device_ids:
            ids = (ctypes.c_int64 * len(device_ids))(*device_ids)
            rc = lib.axon_start_nrt_profile(ids, len(device_ids))
        else:
            rc = lib.axon_start_nrt_profile(None, 0)
        if rc != 0:
            raise RuntimeError(f"axon_start_nrt_profile rc={rc}")
        try:
            yield
        finally:
            n = lib.axon_stop_nrt_profile(str(output_dir).encode())
            print(f"profile: {n} file(s) written to {output_dir}")

    mod.set_axon_ntff_profile_hook(_hook)

    # Avoid the bucket upload inside the trace post-processing.
    import concourse.bass_utils as bu

    bu.upload_artifacts = lambda tmpdir: f"local:{tmpdir}"


def kernel(u, W):
    from concourse.bass_utils import run_bass_kernel_spmd

    if os.environ.get("KERNEL_TRACE", "0") == "1":
        _install_profile_hook()
    if "nc" not in _CACHE:
        _CACHE["nc"] = _build_program()
    nc = _CACHE["nc"]
    in_maps = _host_inputs(u, W)
    trace = os.environ.get("KERNEL_TRACE", "0") == "1"
    res = run_bass_kernel_spmd(
        nc, in_maps, core_ids=list(range(NCORES)), trace=trace
    )
    _CACHE["last_result"] = res
    out = np.zeros((B, CD), dtype=np.float32)
    for ci in range(NCORES):
        sl = np.asarray(res.results[ci]["out"])    # (2*PSLC, CD)
        out[ci * PSLC:(ci + 1) * PSLC] = sl[:PSLC]
        out[128 + ci * PSLC:128 + (ci + 1) * PSLC] = sl[PSLC:]
    return out.reshape(B, C, D)
